# revision 4
# baseline (speedup 1.0000x reference)
"""Trainium2 Bass kernel for nn_Controller (ENAS-style LSTM sampling chain).

Structure:
  - 54 sequential LSTM steps (H=64) with data-dependent input selection
    driven by Gumbel-argmax sampling (exact replication of
    jax.random.categorical: host-precomputed gumbel noise + device argmax).
  - Sigmoid computed as 0.5*tanh(x/2)+0.5 (only the resident ACT table set
    works on this runtime; it has tanh). Scaled-state convention C=2c,
    h2=2h folds the post-affine into weights.
  - Softmax/entropy/logprob outputs deferred: per-sample tanh rows stored,
    batch-processed at the end with NR reciprocal + atanh-series ln
    (no exp/ln/sigmoid tables, no division hardware needed).
Replicated on all 8 cores (sequential chain is not shardable); core 0's
output is returned.
"""
import numpy as np

H = 64
NUM_NODES = 7
TANH_C = 1.1
OP_RED = 2.5
NSAMP = 40          # 2 cells x 5 nodes x 4 samples
SLOT = 18           # per-sample slots in store row: t[0:8] oh[8:16] la[16] arc[17]
LN2 = 0.6931471805599453

_CACHE = {}


def _host_constants():
    """Gumbel noise (exact jax.random.categorical draws) + misc constants.
    Input-independent: depends only on the fixed base key 42 and shapes."""
    if "gum" in _CACHE:
        return _CACHE["gum"]
    import jax
    import jax.numpy as jnp
    cpu = jax.devices("cpu")[0]
    with jax.default_device(cpu):
        base = jax.random.key(42)
        # per-sample category counts, chain order (cell-continuous steps)
        ns, kinds = [], []
        for _cell in range(2):
            for node in range(2, NUM_NODES):
                ns += [node, node, 5, 5]
                kinds += ["idx", "idx", "op", "op"]
        gum = np.zeros((1, 8 * NSAMP), np.float32)
        for s in range(NSAMP):
            k = jax.random.fold_in(base, s + 1)
            g = np.asarray(jax.random.gumbel(k, (ns[s],), jnp.float32))
            gum[0, 8 * s:8 * s + ns[s]] = g
    _CACHE["gum"] = (gum, ns, kinds)
    return gum, ns, kinds


def _prep_inputs(inp):
    """Host-side weight packing/scaling for the device kernel."""
    gum, ns, kinds = _host_constants()
    w_ih = np.asarray(inp["w_ih"], np.float32)
    w_hh = np.asarray(inp["w_hh"], np.float32)
    add_bias = np.asarray(inp["additional_bias"], np.float32)

    def gate_cols(w, rows):
        # lhsT [64(K), 64(M)] = 0.5 * w[rows, :].T
        return 0.5 * w[rows, :].T.astype(np.float32)

    # gate row blocks in w_ih/w_hh: i=0:64 f=64:128 g=128:192 o=192:256
    # col0 partitions: [f; i], col1: [o; g]
    lx0 = np.concatenate([gate_cols(w_ih, slice(64, 128)),
                          gate_cols(w_ih, slice(0, 64))], axis=1)     # [64,128]
    lh0 = np.concatenate([gate_cols(w_hh, slice(64, 128)),
                          gate_cols(w_hh, slice(0, 64))], axis=1)
    lx1 = np.concatenate([gate_cols(w_ih, slice(192, 256)),
                          gate_cols(w_ih, slice(128, 192))], axis=1)
    lh1 = np.concatenate([gate_cols(w_hh, slice(192, 256)),
                          gate_cols(w_hh, slice(128, 192))], axis=1)

    wicT = 0.5 * np.asarray(inp["w_index_curr"], np.float32).T           # [64,64]
    wipT = 0.5 * np.asarray(inp["w_index_prev"], np.float32).T           # [64,64]
    wioT = np.asarray(inp["w_index_out"], np.float32).T.copy()           # [64,1]
    wopsT = 0.5 * np.asarray(inp["w_ops"], np.float32).T.copy()          # [64,5]
    bops = np.asarray(inp["b_ops"], np.float32).reshape(1, 5).copy()
    eops2 = 2.0 * np.asarray(inp["embed_ops_w"], np.float32)             # [5,64]
    ef2 = 2.0 * np.asarray(inp["embed_first_w"], np.float32).T.copy()    # [64,1]
    i64 = np.eye(64, dtype=np.float32)
    ejs = np.zeros((1, 49), np.float32)
    for j in range(7):
        ejs[0, 7 * j + j] = 1.0
    one = np.ones((1, 1), np.float32)
    idxrow = np.arange(8, dtype=np.float32).reshape(1, 8)

    # gumbel row: op samples get gumbel + additional_bias folded in
    gmod = gum.copy()
    for s in range(NSAMP):
        if kinds[s] == "op":
            gmod[0, 8 * s:8 * s + 5] += add_bias

    scalecol = np.array([[TANH_C if k == "idx" else TANH_C / OP_RED]
                         for k in kinds], np.float32)                    # [40,1]
    abmat = np.full((NSAMP, 8), -1e30, np.float32)
    abclean = np.zeros((NSAMP, 8), np.float32)
    for s in range(NSAMP):
        n = ns[s]
        abmat[s, :n] = 0.0
        if kinds[s] == "op":
            abmat[s, :5] = add_bias
            abclean[s, :5] = add_bias

    return {
        "lx0": lx0, "lh0": lh0, "lx1": lx1, "lh1": lh1,
        "wicT": wicT, "wipT": wipT, "wioT": wioT, "wopsT": wopsT,
        "bops": bops, "eops2": eops2, "ef2": ef2, "i64": i64,
        "ejs": ejs, "one": one, "idxrow": idxrow, "gum": gmod,
        "scalecol": scalecol, "abmat": abmat, "abclean": abclean,
    }, ns, kinds


def build_nc():
    import concourse.bacc as bacc
    import concourse.mybir as mybir
    import concourse.tile as tile

    DT = mybir.dt.float32
    AF = mybir.ActivationFunctionType
    OP = mybir.AluOpType
    AX = mybir.AxisListType

    _, ns, kinds = _host_constants()

    nc = bacc.Bacc()
    P = nc.declare_dram_parameter
    ins = {}
    for name, shape in [
            ("lx0", [64, 128]), ("lh0", [64, 128]), ("lx1", [64, 128]),
            ("lh1", [64, 128]), ("wicT", [64, 64]), ("wipT", [64, 64]),
            ("wioT", [64, 1]), ("wopsT", [64, 5]), ("bops", [1, 5]),
            ("eops2", [5, 64]), ("ef2", [64, 1]), ("i64", [64, 64]),
            ("ejs", [1, 49]), ("one", [1, 1]), ("idxrow", [1, 8]),
            ("gum", [1, 8 * NSAMP]), ("scalecol", [NSAMP, 1]),
            ("abmat", [NSAMP, 8]), ("abclean", [NSAMP, 8])]:
        ins[name] = P(name, shape, DT, isOutput=False)
    o_arc = P("o_arc", [NSAMP, 1], DT, isOutput=True)
    o_ents = P("o_ents", [NSAMP, 1], DT, isOutput=True)
    o_lps = P("o_lps", [NSAMP, 1], DT, isOutput=True)

    with tile.TileContext(nc) as tc:
        with tc.tile_pool(name="w", bufs=1) as wp, \
             tc.tile_pool(name="st", bufs=1) as sp, \
             tc.tile_pool(name="ps", bufs=1, space="PSUM") as pp:
            # ---- load constants ----
            t = {}
            for name in ins:
                shape = list(ins[name].shape)
                t[name] = wp.tile(shape, DT, tag=name, name=f"c_{name}")
                nc.sync.dma_start(out=t[name][:, :], in_=ins[name][:, :])

            # ---- persistent state ----
            h2t = sp.tile([64, 1], DT, tag="h2t")
            C = sp.tile([64, 1], DT, tag="C")              # 2*c
            prevH = sp.tile([7, 64], DT, tag="prevH")
            prevFC = sp.tile([64, 7], DT, tag="prevFC")
            x2 = sp.tile([64, 1], DT, tag="x2")
            lg_sb = sp.tile([64, 1], DT, tag="lg_sb")
            qs = sp.tile([64, 6], DT, tag="qs")
            q = sp.tile([64, 6], DT, tag="q")
            tgh = sp.tile([128, 1], DT, tag="tgh")   # tg at [64:128]
            sf = sp.tile([64, 1], DT, tag="sf")
            so = sp.tile([64, 1], DT, tag="so")
            trow = sp.tile([1, 8], DT, tag="trow")
            zrow = sp.tile([1, 8], DT, tag="zrow")
            mrow = sp.tile([1, 1], DT, tag="mrow")
            ohrow = sp.tile([1, 8], DT, tag="ohrow")
            ohcol = sp.tile([8, 1], DT, tag="ohcol")
            h2row = sp.tile([1, 64], DT, tag="h2row")
            store = sp.tile([1, SLOT * NSAMP], DT, tag="store")

            gates = pp.tile([128, 2], DT, tag="gates")
            st0 = pp.tile([128, 1], DT, tag="st0")
            T1p = pp.tile([128, 1], DT, tag="T1p")   # T1 at [64:128]
            tcp = pp.tile([64, 1], DT, tag="tcp")
            misc = pp.tile([64, 64], DT, tag="misc")   # lg/fc/x2/q2/ohT
            h2rp = pp.tile([1, 64], DT, tag="h2rp")
            prevHp = pp.tile([7, 64], DT, tag="prevHp")

            nc.vector.memset(h2t[:, :], 0.0)
            nc.vector.memset(C[:, :], 0.0)
            nc.vector.memset(prevH[:, :], 0.0)
            nc.vector.memset(store[:, :], 0.0)
            nc.vector.memset(ohrow[:, :], 0.0)

            h2 = h2t[0:64, 0:1]

            def lstm_step(x_rhs):
                # gates col0=[f;i], col1=[g;o]; rhs x2-part + h2-part
                nc.tensor.matmul(gates[:, 0:1], t["lx0"][:, :], x_rhs,
                                 start=True, stop=False)
                nc.tensor.matmul(gates[:, 0:1], t["lh0"][:, :], h2,
                                 start=False, stop=True)
                nc.tensor.matmul(gates[:, 1:2], t["lx1"][:, :], x_rhs,
                                 start=True, stop=False)
                nc.tensor.matmul(gates[:, 1:2], t["lh1"][:, :], h2,
                                 start=False, stop=True)
                nc.scalar.activation(st0[:, :], gates[:, 0:1], AF.Tanh,
                                     scale=0.5)
                nc.scalar.activation(so[:, :], gates[0:64, 1:2], AF.Tanh,
                                     scale=0.5)
                nc.scalar.activation(tgh[64:128, :], gates[64:128, 1:2],
                                     AF.Tanh)
                # sf = 0.5*sf' + 0.5 ; T1 = tg*si' + tg ; C = C*sf + T1
                nc.vector.tensor_scalar(out=sf[:, :], in0=st0[0:64, :],
                                        scalar1=0.5, scalar2=0.5,
                                        op0=OP.mult, op1=OP.add)
                nc.vector.scalar_tensor_tensor(
                    out=T1p[64:128, :], in0=tgh[64:128, :],
                    scalar=st0[64:128, :], in1=tgh[64:128, :],
                    op0=OP.mult, op1=OP.add)
                nc.vector.scalar_tensor_tensor(
                    out=C[:, :], in0=C[:, :], scalar=sf[:, :],
                    in1=T1p[64:128, :], op0=OP.mult, op1=OP.add)
                nc.scalar.activation(tcp[:, :], C[:, :], AF.Tanh, scale=0.5)
                nc.vector.scalar_tensor_tensor(
                    out=h2, in0=so[:, :], scalar=1.0,
                    in1=tcp[:, :], op0=OP.add, op1=OP.mult)

            def write_fc(j):
                fcp = misc[0:64, 0:1]
                nc.tensor.matmul(fcp, t["wipT"][:, :], h2,
                                 start=True, stop=True)
                nc.vector.tensor_copy(prevFC[:, j:j + 1], fcp)

            def sample_common(s, n):
                # z -> argmax onehot -> stores; assumes trow has tanh vals
                nc.vector.scalar_tensor_tensor(
                    out=zrow[0:1, 0:n], in0=trow[0:1, 0:n],
                    scalar=float(TANH_C if kinds[s] == "idx"
                                 else TANH_C / OP_RED),
                    in1=t["gum"][0:1, 8 * s:8 * s + n],
                    op0=OP.mult, op1=OP.add)
                nc.vector.tensor_reduce(mrow[:, :], zrow[0:1, 0:n],
                                        axis=AX.X, op=OP.max)
                nc.vector.tensor_scalar(out=ohrow[0:1, 0:n],
                                        in0=zrow[0:1, 0:n],
                                        scalar1=mrow[:, :], scalar2=None,
                                        op0=OP.is_equal)
                ohT = misc[0:n, 1:2]
                nc.tensor.matmul(ohT, ohrow[0:1, 0:n], t["one"][:, :],
                                 start=True, stop=True)
                nc.vector.tensor_copy(ohcol[0:n, :], ohT)
                # stores (off critical path)
                base = SLOT * s
                nc.vector.tensor_copy(store[0:1, base:base + n],
                                      trow[0:1, 0:n])
                nc.vector.tensor_copy(store[0:1, base + 8:base + 8 + n],
                                      ohrow[0:1, 0:n])
                nc.vector.scalar_tensor_tensor(
                    out=zrow[0:1, 0:n], in0=trow[0:1, 0:n], scalar=1.0,
                    in1=ohrow[0:1, 0:n], op0=OP.mult, op1=OP.mult,
                    accum_out=store[0:1, base + 16:base + 17])
                nc.vector.scalar_tensor_tensor(
                    out=zrow[0:1, 0:n], in0=t["idxrow"][0:1, 0:n],
                    scalar=1.0, in1=ohrow[0:1, 0:n],
                    op0=OP.mult, op1=OP.mult,
                    accum_out=store[0:1, base + 17:base + 18])

            def index_sample(s, n):
                lgp = misc[0:64, 2:3]
                nc.tensor.matmul(lgp, t["wicT"][:, :], h2,
                                 start=True, stop=True)
                nc.vector.tensor_copy(lg_sb[:, :], lgp)
                nc.vector.tensor_scalar(out=qs[:, 0:n],
                                        in0=prevFC[:, 0:n],
                                        scalar1=lg_sb[:, :], scalar2=None,
                                        op0=OP.add)
                nc.scalar.activation(q[:, 0:n], qs[:, 0:n], AF.Tanh)
                q2 = misc[0:1, 3:3 + n]
                nc.tensor.matmul(q2, t["wioT"][:, :], q[:, 0:n],
                                 start=True, stop=True)
                nc.scalar.activation(trow[0:1, 0:n], q2, AF.Tanh)
                sample_common(s, n)
                x2p = misc[0:64, 10:11]
                nc.tensor.matmul(x2p, prevH[0:n, :], ohcol[0:n, :],
                                 start=True, stop=True)
                nc.vector.tensor_copy(x2[:, :], x2p)

            def op_sample(s):
                olp = misc[0:1, 11:16]
                nc.tensor.matmul(olp, h2, t["wopsT"][:, :],
                                 start=True, stop=False)
                nc.tensor.matmul(olp, t["one"][:, :], t["bops"][:, :],
                                 start=False, stop=True)
                nc.scalar.activation(trow[0:1, 0:5], olp, AF.Tanh,
                                     scale=float(1.0 / OP_RED))
                sample_common(s, 5)
                x2p = misc[0:64, 16:17]
                nc.tensor.matmul(x2p, t["eops2"][:, :], ohcol[0:5, :],
                                 start=True, stop=True)
                nc.vector.tensor_copy(x2[:, :], x2p)

            def finalize(j, first):
                nc.tensor.matmul(h2rp[:, :], h2, t["i64"][:, :],
                                 start=True, stop=True)
                nc.vector.tensor_copy(h2row[:, :], h2rp[:, :])
                nc.tensor.matmul(prevHp[:, :], t["ejs"][0:1, 7 * j:7 * j + 7],
                                 h2row[:, :], start=first, stop=True,
                                 skip_group_check=True)
                nc.vector.tensor_copy(prevH[:, :], prevHp[:, :])
                write_fc(j)

            # ================= the chain =================
            s = 0
            for cell in range(2):
                nc.vector.memset(prevH[:, :], 0.0)
                for j in range(2):
                    lstm_step(t["ef2"][:, :])
                    write_fc(j)
                for node in range(2, NUM_NODES):
                    for k in range(2):
                        lstm_step(t["ef2"][:, :] if (node == 2 and k == 0)
                                  or (node > 2 and k == 0) else x2[:, :])
                        index_sample(s, node)
                        s += 1
                    for k in range(2):
                        lstm_step(x2[:, :])
                        op_sample(s)
                        s += 1
                    lstm_step(x2[:, :])
                    finalize(node, first=(node == 2))

            # ================= batch softmax phase =================
            smat = sp.tile([NSAMP, SLOT], DT, tag="smat")
            nc.sync.dma_start(out=smat[:, :], in_=store[0:1, :])

            B = NSAMP
            bp = sp
            logits = bp.tile([B, 8], DT, tag="logits")
            mcol = bp.tile([B, 1], DT, tag="mcol")
            lsh = bp.tile([B, 8], DT, tag="lsh")
            th = bp.tile([B, 8], DT, tag="th")
            num = bp.tile([B, 8], DT, tag="num")
            den = bp.tile([B, 8], DT, tag="den")
            r = bp.tile([B, 8], DT, tag="r")
            u8 = bp.tile([B, 8], DT, tag="u8")
            e = bp.tile([B, 8], DT, tag="e")
            scol = bp.tile([B, 1], DT, tag="scol")
            c1 = bp.tile([B, 1], DT, tag="c1")
            c2 = bp.tile([B, 1], DT, tag="c2")
            s2 = bp.tile([B, 1], DT, tag="s2")
            rb = bp.tile([B, 1], DT, tag="rb")
            ub = bp.tile([B, 1], DT, tag="ub")
            u2 = bp.tile([B, 1], DT, tag="u2")
            pol = bp.tile([B, 1], DT, tag="pol")
            lnS = bp.tile([B, 1], DT, tag="lnS")
            dot = bp.tile([B, 1], DT, tag="dot")
            scr = bp.tile([B, 8], DT, tag="scr")
            lacol = bp.tile([B, 1], DT, tag="lacol")
            absel = bp.tile([B, 1], DT, tag="absel")
            entc = bp.tile([B, 1], DT, tag="entc")
            lpsc = bp.tile([B, 1], DT, tag="lpsc")

            TS, STT, TT = (nc.vector.tensor_scalar,
                           nc.vector.scalar_tensor_tensor,
                           nc.vector.tensor_tensor)

            # logits = t*scale + abmat  (pads -> -1e30)
            STT(out=logits[:, :], in0=smat[:, 0:8],
                scalar=t["scalecol"][:, 0:1], in1=t["abmat"][:, :],
                op0=OP.mult, op1=OP.add)
            nc.vector.tensor_reduce(mcol[:, :], logits[:, :],
                                    axis=AX.X, op=OP.max)
            TS(out=lsh[:, :], in0=logits[:, :], scalar1=mcol[:, :],
               scalar2=None, op0=OP.subtract)
            nc.scalar.activation(th[:, :], lsh[:, :], AF.Tanh, scale=0.5)
            TS(out=num[:, :], in0=th[:, :], scalar1=1.0, scalar2=None,
               op0=OP.add)
            TS(out=den[:, :], in0=th[:, :], scalar1=1.0, scalar2=-1.0,
               op0=OP.subtract, op1=OP.mult)   # (th-1)*-1 = 1-th in [1,2]

            def recip(dst, src, shape_cols, c1v, c2v, scratch):
                TS(out=dst, in0=src, scalar1=c2v, scalar2=c1v,
                   op0=OP.mult, op1=OP.subtract)          # src*c2 - c1
                TS(out=dst, in0=dst, scalar1=-1.0, scalar2=None,
                   op0=OP.mult)                            # c1 - c2*src
                for _ in range(3):
                    TT(out=scratch, in0=src, in1=dst, op=OP.mult)
                    TS(out=scratch, in0=scratch, scalar1=2.0, scalar2=-1.0,
                       op0=OP.subtract, op1=OP.mult)       # 2 - d*r
                    TT(out=dst, in0=dst, in1=scratch, op=OP.mult)

            recip(r[:, :], den[:, :], 8, 24.0 / 17.0, 8.0 / 17.0, scr[:, :])
            TT(out=e[:, :], in0=num[:, :], in1=r[:, :], op=OP.mult)
            nc.vector.tensor_reduce(scol[:, :], e[:, :], axis=AX.X, op=OP.add)

            # range-reduce s in [1,6] -> s2 in [1,2]; lnS = lnm + k*ln2
            TS(out=c1[:, :], in0=scol[:, :], scalar1=2.0, scalar2=None,
               op0=OP.is_ge)
            TS(out=c2[:, :], in0=scol[:, :], scalar1=4.0, scalar2=None,
               op0=OP.is_ge)
            TS(out=s2[:, :], in0=c1[:, :], scalar1=-0.5, scalar2=1.0,
               op0=OP.mult, op1=OP.add)
            STT(out=u2[:, :], in0=c2[:, :], scalar=-0.5, in1=s2[:, :],
                op0=OP.mult, op1=OP.mult)   # u2 misused as tmp: (c2*-0.5)*s2
            TT(out=u2[:, :], in0=u2[:, :], in1=s2[:, :], op=OP.add)
            # now u2 = s2*(1-0.5*c2) -> factor f applied to scol:
            TT(out=s2[:, :], in0=u2[:, :], in1=scol[:, :], op=OP.mult)
            # s2 = scol * f  where f = (1-0.5c1)(1-0.5c2)

            # recip(s2) on [1,2] and recip(s2+1) on [2,3]
            recip(rb[:, :], s2[:, :], 1, 24.0 / 17.0, 8.0 / 17.0, ub[:, :])
            TS(out=ub[:, :], in0=s2[:, :], scalar1=1.0, scalar2=None,
               op0=OP.add)                                  # s2+1
            recip(u2[:, :], ub[:, :], 1, 5.0 / 6.0, 1.0 / 6.0, pol[:, :])
            TS(out=ub[:, :], in0=s2[:, :], scalar1=1.0, scalar2=None,
               op0=OP.subtract)                             # s2-1
            TT(out=ub[:, :], in0=ub[:, :], in1=u2[:, :], op=OP.mult)  # u
            TT(out=u2[:, :], in0=ub[:, :], in1=ub[:, :], op=OP.mult)  # u^2
            TS(out=pol[:, :], in0=u2[:, :], scalar1=1.0 / 13.0,
               scalar2=1.0 / 11.0, op0=OP.mult, op1=OP.add)
            for cc in (1.0 / 9.0, 1.0 / 7.0, 1.0 / 5.0, 1.0 / 3.0, 1.0):
                TS(out=pol[:, :], in0=u2[:, :], scalar1=pol[:, :],
                   scalar2=cc, op0=OP.mult, op1=OP.add)
            TS(out=pol[:, :], in0=ub[:, :], scalar1=pol[:, :], scalar2=2.0,
               op0=OP.mult, op1=OP.mult)                    # ln(s2)
            TT(out=lnS[:, :], in0=c1[:, :], in1=c2[:, :], op=OP.add)
            TS(out=lnS[:, :], in0=lnS[:, :], scalar1=LN2, scalar2=None,
               op0=OP.mult)
            TT(out=lnS[:, :], in0=lnS[:, :], in1=pol[:, :], op=OP.add)

            # recip(scol) = rb * f: f = s2/scol ... instead rb is 1/s2;
            # 1/scol = rb * (s2/scol) = rb * f. Recover f = s2*rscol? Avoid:
            # f = (1-0.5c1)(1-0.5c2): rebuild cheaply
            TS(out=u2[:, :], in0=c1[:, :], scalar1=-0.5, scalar2=1.0,
               op0=OP.mult, op1=OP.add)
            TS(out=ub[:, :], in0=c2[:, :], scalar1=-0.5, scalar2=1.0,
               op0=OP.mult, op1=OP.add)
            TT(out=u2[:, :], in0=u2[:, :], in1=ub[:, :], op=OP.mult)  # f
            TT(out=rb[:, :], in0=rb[:, :], in1=u2[:, :], op=OP.mult)  # 1/scol

            # ents = lnS - (sum e*lsh) / s
            STT(out=scr[:, :], in0=e[:, :], scalar=1.0, in1=lsh[:, :],
                op0=OP.mult, op1=OP.mult, accum_out=dot[:, :])
            TT(out=dot[:, :], in0=dot[:, :], in1=rb[:, :], op=OP.mult)
            TT(out=entc[:, :], in0=lnS[:, :], in1=dot[:, :], op=OP.subtract)

            # lps = la - m - lnS ; la = laraw*scale + absel
            STT(out=scr[:, :], in0=smat[:, 8:16], scalar=1.0,
                in1=t["abclean"][:, :], op0=OP.mult, op1=OP.mult,
                accum_out=absel[:, :])
            STT(out=lacol[:, :], in0=smat[:, 16:17],
                scalar=t["scalecol"][:, 0:1], in1=absel[:, :],
                op0=OP.mult, op1=OP.add)
            TT(out=lpsc[:, :], in0=lacol[:, :], in1=mcol[:, :],
               op=OP.subtract)
            TT(out=lpsc[:, :], in0=lpsc[:, :], in1=lnS[:, :],
               op=OP.subtract)

            nc.sync.dma_start(out=o_arc[:, :], in_=smat[:, 17:18])
            nc.sync.dma_start(out=o_ents[:, :], in_=entc[:, :])
            nc.sync.dma_start(out=o_lps[:, :], in_=lpsc[:, :])
    nc.finalize()
    return nc


def kernel(**inputs):
    from concourse import bass_utils
    consts, ns, kinds = _prep_inputs(inputs)
    if "nc" not in _CACHE:
        _CACHE["nc"] = build_nc()
    nc = _CACHE["nc"]
    in_map = {k: np.ascontiguousarray(v) for k, v in consts.items()}
    res = bass_utils.run_bass_kernel_spmd(
        nc, [in_map] * 8, core_ids=list(range(8)))
    r = res.results[0]
    arc = np.rint(r["o_arc"].reshape(-1)).astype(np.int32)
    ents = r["o_ents"].reshape(-1).astype(np.float32)
    lps = r["o_lps"].reshape(-1).astype(np.float32)
    return arc[0:20], arc[20:40], ents, lps
